# revision 1
# baseline (speedup 1.0000x reference)
"""GATWithSentenceEmbedding Trainium2 kernel (8 NeuronCores, SPMD + collectives).

Sharding:
  - fcl [E,E] / fce [BERT,E] column-sharded (each core computes a 1024-chunk of
    g1/g2); fc2 [2E,E] row-sharded with matching rows; one AllReduce yields the
    full orig_edge_logits on every core.
  - GAT: edges sorted by dst; core c owns dst nodes [256c, 256c+256) and their
    incoming edges (incl. self-loops). Segment softmax/aggregation via one-hot
    matmul into PSUM. xp2 / h2 chunks are AllGathered between layers.
  - Edge MLP: same dst-based edge partition (real edges only); masked-BN stats
    combined with two tiny AllReduces.
"""

import numpy as np
from contextlib import ExitStack

import concourse.bass as bass
import concourse.mybir as mybir
import concourse.tile as tile
from concourse import bacc
from concourse.bass_utils import run_bass_kernel_spmd
from concourse.masks import make_identity

N, F, HC, S, H, E, BERT = 2048, 256, 256, 512, 4, 8192, 768
NCORES = 8
P = 128
NCHUNK = N // NCORES          # 256 dst nodes per core
ECH = E // NCORES             # 1024 g1/g2 columns per core
XP1W = H * HC + 2 * H         # 1032 = xp1 | al_s | al_d
XP2W = F + 2                  # 258  = xp2 | al_s | al_d
HC2 = HC // 2                 # 128
BIG = 1.0e9

dt = mybir.dt
AF = mybir.ActivationFunctionType
ALU = mybir.AluOpType
RG = [list(range(NCORES))]

_cache = {}
last_in_maps = None
DEBUG = False
TRACE = False
last_results = None


def _build(nt_g: int, nt_m: int, debug: bool = False, stage: int = 4):
    pad_g = nt_g * P
    pad_m = nt_m * P
    nc = bacc.Bacc("TRN2", target_bir_lowering=False, debug=False)

    def inp(name, shape, dtype=dt.float32):
        return nc.dram_tensor(name, shape, dtype, kind="ExternalInput")

    # shared inputs
    x_in = inp("x", [N, F])
    sent_in = inp("sent_emb", [BERT])
    elp_in = inp("elp", [E])
    fc0_w = inp("fc0_w", [BERT, S]); fc0_b = inp("fc0_b", [S])
    fc1_w = inp("fc1_w", [F, S]); fc1_b = inp("fc1_b", [S])
    c1w = inp("conv1_W", [S, H * HC]); c1a = inp("conv1_a", [2 * H * HC])
    c1b = inp("conv1_b", [H * HC])
    c2w = inp("conv2_W", [H * HC, F]); c2a = inp("conv2_a", [2 * F])
    c2b = inp("conv2_b", [F])
    m1w = inp("mlp1_w", [4 * F, HC]); m1b = inp("mlp1_b", [HC])
    bn1g = inp("bn1_g", [HC]); bn1b = inp("bn1_b", [HC])
    m2w = inp("mlp2_w", [HC, HC2]); m2b = inp("mlp2_b", [HC2])
    bn2g = inp("bn2_g", [HC2]); bn2b = inp("bn2_b", [HC2])
    m3w = inp("mlp3_w", [HC2, 1]); m3b = inp("mlp3_b", [1])
    fc2_b = inp("fc2_b", [E])
    # per-core inputs
    fclw_sh = inp("fclw_sh", [E, ECH]); fclb_sh = inp("fclb_sh", [ECH])
    fcew_sh = inp("fcew_sh", [BERT, ECH]); fceb_sh = inp("fceb_sh", [ECH])
    fc2w_sh = inp("fc2w_sh", [2 * ECH, E])
    g_src = inp("g_src", [pad_g], dt.int32)
    g_dst = inp("g_dst", [pad_g], dt.int32)
    g_lidx = inp("g_lidx", [pad_g], dt.int32)
    g_oh = inp("g_oh", [pad_g, NCHUNK], dt.float16)
    m_src = inp("m_src", [pad_m], dt.int32)
    m_dst = inp("m_dst", [pad_m], dt.int32)
    m_lidx = inp("m_lidx", [pad_m], dt.int32)
    # outputs
    orig_out = nc.dram_tensor("orig_out", [E], dt.float32, kind="ExternalOutput")
    score_out = nc.dram_tensor("score_out", [pad_m], dt.float32,
                               kind="ExternalOutput")
    dbg = {}
    if debug:
        for nm, shp in [("h_dbg", [N, S]), ("xp1_dbg", [N, XP1W]),
                        ("h1_dbg", [NCHUNK, H * HC]), ("xp2_dbg", [N, XP2W]),
                        ("h2_dbg", [N, F]), ("z1_dbg", [pad_m, HC]),
                        ("st1_dbg", [520]),
                        ("xs_dbg", [pad_g, XP1W]), ("ad_dbg", [pad_g, 2 * H]),
                        ("ex_dbg", [pad_g, H]), ("den_dbg", [2 * P, H]),
                        ("msum_dbg", [2 * P, H * HC])]:
            dbg[nm] = nc.dram_tensor(nm, shp, dt.float32, kind="ExternalOutput")

    def bcast(dram_handle, cols, offset=0):
        """AP reading a [1, cols] DRAM row replicated over 128 partitions."""
        return bass.AP(tensor=dram_handle.ap().tensor, offset=offset,
                       ap=[[0, P], [1, cols]])

    def bcast_ap(ap_tile, cols, offset=0):
        a = ap_tile[:] if not isinstance(ap_tile, bass.AP) else ap_tile
        return bass.AP(tensor=a.tensor, offset=a.offset + offset,
                       ap=[[0, P], [1, cols]])

    with tile.TileContext(nc) as tc:
        with (
            tc.tile_pool(name="dram", bufs=1, space="DRAM") as dram,
            tc.tile_pool(name="single", bufs=1) as single,
            tc.tile_pool(name="sb", bufs=2) as sb,
            tc.tile_pool(name="psum2", bufs=2, space="PSUM") as psum2,
            tc.tile_pool(name="keep", bufs=1) as keep,
        ):
            ident = single.tile([P, P], dt.float32)
            make_identity(nc, ident[:])
            ident_h = single.tile([P, P], dt.float16)
            nc.vector.tensor_copy(ident_h[:], ident[:])

            # internal DRAM
            xp1_dram = dram.tile([N, XP1W], dt.float16)
            al1d_dram = dram.tile([N, 2 * H], dt.float16)
            fc2part = dram.tile([E], dt.float32)
            logits_dram = dram.tile([E], dt.float32, addr_space="Shared")
            lext_dram = dram.tile([E + 2, 1], dt.float32)
            sent_dram = dram.tile([S], dt.float32)
            g12_dram = dram.tile([2 * ECH], dt.float32)
            xp2_in = dram.tile([NCHUNK, XP2W], dt.float16)
            xp2_dram = dram.tile([N, XP2W], dt.float16, addr_space="Shared")
            h2_in = dram.tile([NCHUNK, F], dt.float16)
            h2_dram = dram.tile([N, F], dt.float16, addr_space="Shared")
            st1_in = dram.tile([520], dt.float32)
            st1_out = dram.tile([520], dt.float32, addr_space="Shared")
            st2_in = dram.tile([2 * HC2], dt.float32)
            st2_out = dram.tile([2 * HC2], dt.float32, addr_space="Shared")
            row_dram = dram.tile([4 * HC], dt.float32)  # scratch rows for bcast

            # ======== phases A (h/xp1) + B (g1/g2/fc2) — scoped pools ========
            esA = ExitStack()
            sbA = esA.enter_context(tc.tile_pool(name="sbA", bufs=2))
            psA = esA.enter_context(tc.tile_pool(name="psA", bufs=1, space="PSUM"))

            # sent = relu(sent_emb @ fc0_w + fc0_b), weights-stationary chunks
            sent_sb = single.tile([P, BERT // P], dt.float32)
            nc.sync.dma_start(sent_sb[:], sent_in.ap().rearrange("(k p) -> p k", p=P))
            fc0w_t = [sbA.tile([P, S], dt.float32, tag=f"fc0w{k}", bufs=1,
                               name=f"fc0w{k}")
                      for k in range(BERT // P)]
            for k in range(BERT // P):
                nc.sync.dma_start(fc0w_t[k][:], fc0_w[k * P:(k + 1) * P, :])
            for j in range(S // P):
                ps_v = psA.tile([P, 1], dt.float32, space="PSUM", tag="vec",
                                bufs=2, name="ps_v")
                for k in range(BERT // P):
                    nc.tensor.matmul(ps_v[:], lhsT=fc0w_t[k][:, j * P:(j + 1) * P],
                                     rhs=sent_sb[:, k:k + 1],
                                     start=(k == 0), stop=(k == BERT // P - 1))
                bcol = sbA.tile([P, 1], dt.float32, tag="bcol")
                nc.sync.dma_start(bcol[:], fc0_b[j * P:(j + 1) * P][:, None])
                sc = sbA.tile([P, 1], dt.float32, tag="scol")
                nc.vector.tensor_add(sc[:], ps_v[:], bcol[:])
                nc.scalar.activation(sc[:], sc[:], AF.Relu)
                nc.sync.dma_start(sent_dram[j * P:(j + 1) * P][:, None], sc[:])
            sent_bc = single.tile([P, S], dt.float32)
            nc.sync.dma_start(sent_bc[:], bcast_ap(sent_dram, S))

            # W1aug = [conv1_W | W@a_src | W@a_dst] as 4 k-tiles [128, 1032]
            c1a_bc = sbA.tile([P, 2 * H * HC], dt.float32, tag="c1abc", bufs=1)
            nc.sync.dma_start(c1a_bc[:], bcast(c1a, 2 * H * HC))
            w1aug = [sbA.tile([P, XP1W], dt.float32, tag=f"w1aug{k}", bufs=1,
                              name=f"w1aug{k}")
                     for k in range(S // P)]
            for k in range(S // P):
                nc.sync.dma_start(w1aug[k][:, 0:H * HC],
                                  c1w[k * P:(k + 1) * P, :])
                tmp = sbA.tile([P, H * HC], dt.float32, tag="scratch4k")
                nc.vector.tensor_mul(tmp[:], w1aug[k][:, 0:H * HC],
                                     c1a_bc[:, 0:H * HC])
                for h in range(H):
                    nc.vector.reduce_sum(
                        w1aug[k][:, H * HC + h:H * HC + h + 1],
                        tmp[:, h * HC:(h + 1) * HC], axis=mybir.AxisListType.X)
                nc.vector.tensor_mul(tmp[:], w1aug[k][:, 0:H * HC],
                                     c1a_bc[:, H * HC:2 * H * HC])
                for h in range(H):
                    nc.vector.reduce_sum(
                        w1aug[k][:, H * HC + H + h:H * HC + H + h + 1],
                        tmp[:, h * HC:(h + 1) * HC], axis=mybir.AxisListType.X)

            # h = relu(x @ fc1_w + fc1_b) + sent ; xp1aug = h @ W1aug
            fc1w_t = [sbA.tile([P, S], dt.float32, tag=f"fc1w{k}", bufs=1,
                               name=f"fc1w{k}")
                      for k in range(F // P)]
            for k in range(F // P):
                nc.sync.dma_start(fc1w_t[k][:], fc1_w[k * P:(k + 1) * P, :])
            w1aug_h = [sbA.tile([P, XP1W], dt.float16, tag=f"w1augh{k}", bufs=1,
                               name=f"w1augh{k}")
                       for k in range(S // P)]
            for k in range(S // P):
                nc.vector.tensor_copy(w1aug_h[k][:], w1aug[k][:])
            fc1b_bc = sbA.tile([P, S], dt.float32, tag="fc1bbc", bufs=1)
            nc.sync.dma_start(fc1b_bc[:], bcast(fc1_b, S))
            for nt in range(N // P):
                x_t = sbA.tile([P, F], dt.float32, tag="x")
                nc.sync.dma_start(x_t[:], x_in[nt * P:(nt + 1) * P, :])
                ps_h = psA.tile([P, S], dt.float32, space="PSUM", tag="ps_h")
                for k in range(F // P):
                    ps_xt = psum2.tile([P, P], dt.float32, space="PSUM", tag="ps_xt")
                    nc.tensor.transpose(ps_xt[:], x_t[:, k * P:(k + 1) * P], ident[:])
                    xT = sb.tile([P, P], dt.float32, tag="xT")
                    nc.vector.tensor_copy(xT[:], ps_xt[:])
                    nc.tensor.matmul(ps_h[:], lhsT=xT[:], rhs=fc1w_t[k][:],
                                     start=(k == 0), stop=(k == F // P - 1))
                h_t = sbA.tile([P, S], dt.float32, tag="h")
                nc.vector.tensor_add(h_t[:], ps_h[:], fc1b_bc[:])
                nc.scalar.activation(h_t[:], h_t[:], AF.Relu)
                nc.vector.tensor_add(h_t[:], h_t[:], sent_bc[:])
                if debug:
                    nc.sync.dma_start(dbg["h_dbg"][nt * P:(nt + 1) * P, :], h_t[:])
                ps_xp1 = psA.tile([P, XP1W], dt.float32, space="PSUM", tag="ps_xp1")
                for k in range(S // P):
                    ps_ht = psum2.tile([P, P], dt.float32, space="PSUM", tag="ps_xt")
                    nc.tensor.transpose(ps_ht[:], h_t[:, k * P:(k + 1) * P], ident[:])
                    hT = sb.tile([P, P], dt.float16, tag="xTh")
                    nc.vector.tensor_copy(hT[:], ps_ht[:])
                    for s0, s1 in ((0, 512), (512, 1024), (1024, XP1W)):
                        nc.tensor.matmul(ps_xp1[:, s0:s1], lhsT=hT[:],
                                         rhs=w1aug_h[k][:, s0:s1],
                                         start=(k == 0), stop=(k == S // P - 1))
                xp1_t = sbA.tile([P, XP1W], dt.float16, tag="xp1")
                nc.vector.tensor_copy(xp1_t[:], ps_xp1[:])
                nc.sync.dma_start(xp1_dram[nt * P:(nt + 1) * P, :], xp1_t[:])
                nc.sync.dma_start(al1d_dram[nt * P:(nt + 1) * P, :],
                                  xp1_t[:, H * HC:H * HC + 2 * H])
                if debug:
                    nc.sync.dma_start(dbg["xp1_dbg"][nt * P:(nt + 1) * P, :],
                                      xp1_t[:])

            # g1/g2/fc2 via weights-stationary vector-matmuls (PE ~= LS rate).
            elp_sb = single.tile([P, E // P], dt.float32)
            nc.sync.dma_start(elp_sb[:], elp_in.ap().rearrange("(k p) -> p k", p=P))
            g_sb = single.tile([P, 2 * ECH // P], dt.float32)
            KQ = 16  # k-tiles per column-slice DMA
            for j in range(ECH // P):       # g1 output chunks
                ps_v = psA.tile([P, 1], dt.float32, space="PSUM", tag="vec",
                                bufs=2, name="ps_v")
                for q in range(E // P // KQ):
                    wq = sbA.tile([P, KQ, P], dt.float32, tag="wcol", bufs=3,
                                  name="wq")
                    nc.sync.dma_start(
                        wq[:],
                        fclw_sh[q * KQ * P:(q + 1) * KQ * P,
                                j * P:(j + 1) * P].rearrange(
                                    "(k p) j -> p k j", p=P))
                    for kk in range(KQ):
                        k = q * KQ + kk
                        nc.tensor.matmul(ps_v[:], lhsT=wq[:, kk, :],
                                         rhs=elp_sb[:, k:k + 1],
                                         start=(k == 0), stop=(k == E // P - 1))
                bcol = sbA.tile([P, 1], dt.float32, tag="bcol")
                nc.sync.dma_start(bcol[:], fclb_sh[j * P:(j + 1) * P][:, None])
                nc.vector.tensor_add(g_sb[:, j:j + 1], ps_v[:], bcol[:])
                nc.scalar.activation(g_sb[:, j:j + 1], g_sb[:, j:j + 1], AF.Relu)
            for j in range(ECH // P):       # g2 output chunks
                ps_v = psA.tile([P, 1], dt.float32, space="PSUM", tag="vec",
                                bufs=2, name="ps_v")
                wq = sbA.tile([P, BERT // P, P], dt.float32, tag="wcol",
                              bufs=3, name="wq")
                nc.sync.dma_start(
                    wq[:],
                    fcew_sh[:, j * P:(j + 1) * P].rearrange(
                        "(k p) j -> p k j", p=P))
                for k in range(BERT // P):
                    nc.tensor.matmul(ps_v[:], lhsT=wq[:, k, :],
                                     rhs=sent_sb[:, k:k + 1],
                                     start=(k == 0), stop=(k == BERT // P - 1))
                bcol = sbA.tile([P, 1], dt.float32, tag="bcol")
                nc.sync.dma_start(bcol[:], fceb_sh[j * P:(j + 1) * P][:, None])
                jo = ECH // P + j
                nc.vector.tensor_add(g_sb[:, jo:jo + 1], ps_v[:], bcol[:])
                nc.scalar.activation(g_sb[:, jo:jo + 1], g_sb[:, jo:jo + 1],
                                     AF.Relu)
            # fc2 partial: 64 output chunks, k = 16 g-columns
            for j in range(E // P):
                ps_v = psA.tile([P, 1], dt.float32, space="PSUM", tag="vec",
                                bufs=2, name="ps_v")
                wq = sbA.tile([P, 2 * ECH // P, P], dt.float32, tag="wcol",
                              bufs=3, name="wq")
                nc.sync.dma_start(
                    wq[:],
                    fc2w_sh[:, j * P:(j + 1) * P].rearrange(
                        "(k p) j -> p k j", p=P))
                for k in range(2 * ECH // P):
                    nc.tensor.matmul(ps_v[:], lhsT=wq[:, k, :],
                                     rhs=g_sb[:, k:k + 1],
                                     start=(k == 0), stop=(k == 2 * ECH // P - 1))
                f2c = sbA.tile([P, 1], dt.float32, tag="f2c")
                nc.vector.tensor_copy(f2c[:], ps_v[:])
                nc.sync.dma_start(fc2part[j * P:(j + 1) * P][:, None], f2c[:])
            esA.close()
            if stage >= 2:

                nc.gpsimd.collective_compute(
                    "AllReduce", ALU.add, replica_groups=RG,
                    ins=[fc2part[:]], outs=[logits_dram[:]])
                # logits += fc2_b ; orig_out ; logits_ext
                lg_pf = single.tile([P, E // P], dt.float32)
                nc.sync.dma_start(lg_pf[:], logits_dram[:].rearrange("(p f) -> p f", p=P))
                f2b_pf = single.tile([P, E // P], dt.float32)
                nc.sync.dma_start(f2b_pf[:], fc2_b.ap().rearrange("(p f) -> p f", p=P))
                nc.vector.tensor_add(lg_pf[:], lg_pf[:], f2b_pf[:])
                nc.sync.dma_start(orig_out.ap().rearrange("(p f) -> p f", p=P), lg_pf[:])
                nc.sync.dma_start(
                    lext_dram[0:E, :].rearrange("(p f) x -> p (f x)", p=P), lg_pf[:])
                big_t = single.tile([1, 2], dt.float32)
                nc.vector.memset(big_t[:, 0:1], BIG)
                nc.vector.memset(big_t[:, 1:2], -BIG)
                nc.sync.dma_start(lext_dram[E:E + 2, 0][None, :], big_t[:])

                # ============ conv1 aggregation (per dst-chunk) ============
                gsrc_sb = single.tile([P, nt_g], dt.int32)
                nc.sync.dma_start(gsrc_sb[:], g_src.ap().rearrange("(t p) -> p t", p=P))
                gdst_sb = single.tile([P, nt_g], dt.int32)
                nc.sync.dma_start(gdst_sb[:], g_dst.ap().rearrange("(t p) -> p t", p=P))
                glidx_sb = single.tile([P, nt_g], dt.int32)
                nc.sync.dma_start(glidx_sb[:], g_lidx.ap().rearrange("(t p) -> p t", p=P))
                valid_t = [keep.tile([P, 1], dt.float32, tag=f"valid{t}",
                                     name=f"valid{t}")
                           for t in range(nt_g)]
                esC = ExitStack()
                sbC = esC.enter_context(tc.tile_pool(name="sbC", bufs=2))
                psC = esC.enter_context(tc.tile_pool(name="psC", bufs=1, space="PSUM"))
                ps_msg = [psC.tile([P, H * HC], dt.float32, space="PSUM",
                                   tag=f"ps_msg{d}", name=f"ps_msg{d}")
                          for d in range(2)]
                ps_den = [psC.tile([P, H], dt.float32, space="PSUM",
                                   tag=f"ps_den{d}", name=f"ps_den{d}")
                          for d in range(2)]
                for t in range(nt_g):
                    oh_t = sbC.tile([P, NCHUNK], dt.float16, tag="oh", bufs=6)
                    nc.sync.dma_start(oh_t[:], g_oh[t * P:(t + 1) * P, :])
                    xs = sbC.tile([P, XP1W], dt.float16, tag="gxs", bufs=6)
                    nc.gpsimd.indirect_dma_start(
                        out=xs[:], out_offset=None, in_=xp1_dram[:],
                        in_offset=bass.IndirectOffsetOnAxis(
                            ap=gsrc_sb[:, t:t + 1], axis=0))
                    ad = sbC.tile([P, 2 * H], dt.float16, tag="gad", bufs=6)
                    nc.gpsimd.indirect_dma_start(
                        out=ad[:], out_offset=None, in_=al1d_dram[:],
                        in_offset=bass.IndirectOffsetOnAxis(
                            ap=gdst_sb[:, t:t + 1], axis=0))
                    lg = sbC.tile([P, 1], dt.float32, tag="glg", bufs=6)
                    nc.gpsimd.indirect_dma_start(
                        out=lg[:], out_offset=None, in_=lext_dram[:],
                        in_offset=bass.IndirectOffsetOnAxis(
                            ap=glidx_sb[:, t:t + 1], axis=0))
                    nc.vector.tensor_scalar(valid_t[t][:], lg[:], 0.0, None,
                                            op0=ALU.is_gt)
                    alpha = sbC.tile([P, H], dt.float32, tag="alpha")
                    nc.vector.tensor_add(alpha[:], xs[:, H * HC:H * HC + H],
                                         ad[:, H:2 * H])
                    nc.vector.scalar_tensor_tensor(alpha[:], alpha[:], 0.2, alpha[:],
                                                   op0=ALU.mult, op1=ALU.max)
                    ex = sbC.tile([P, H], dt.float32, tag="ex")
                    nc.scalar.activation(ex[:], alpha[:], AF.Exp)
                    nc.vector.tensor_mul(ex[:], ex[:],
                                         valid_t[t][:].to_broadcast([P, H]))
                    if debug:
                        nc.sync.dma_start(dbg["xs_dbg"][t * P:(t + 1) * P, :], xs[:])
                        nc.sync.dma_start(dbg["ad_dbg"][t * P:(t + 1) * P, :], ad[:])
                        nc.sync.dma_start(dbg["ex_dbg"][t * P:(t + 1) * P, :], ex[:])
                    msg = sbC.tile([P, H * HC + H], dt.float16, tag="msg")
                    for h in range(H):
                        nc.vector.tensor_tensor(
                            msg[:, h * HC:(h + 1) * HC], xs[:, h * HC:(h + 1) * HC],
                            ex[:, h:h + 1].to_broadcast([P, HC]), op=ALU.mult)
                    nc.vector.tensor_copy(msg[:, H * HC:H * HC + H], ex[:])
                    for d in range(2):
                        lhsT = oh_t[:, d * P:(d + 1) * P]
                        st, sp = (t == 0), (t == nt_g - 1)
                        nc.tensor.matmul(ps_msg[d][:, 0:512], lhsT=lhsT,
                                         rhs=msg[:, 0:512], start=st, stop=sp)
                        nc.tensor.matmul(ps_msg[d][:, 512:1024], lhsT=lhsT,
                                         rhs=msg[:, 512:1024], start=st, stop=sp)
                        nc.tensor.matmul(ps_den[d][:], lhsT=lhsT,
                                         rhs=msg[:, H * HC:H * HC + H],
                                         start=st, stop=sp)
                # finalize conv1 + xp2aug
                c1b_bc = sbC.tile([P, H * HC], dt.float32, tag="c1bbc", bufs=1)
                nc.sync.dma_start(c1b_bc[:], bcast(c1b, H * HC))
                c2a_bc = sbC.tile([P, 2 * F], dt.float32, tag="c2abc", bufs=1)
                nc.sync.dma_start(c2a_bc[:], bcast(c2a, 2 * F))
                w2aug = [keep.tile([P, XP2W], dt.float32, tag=f"w2aug{k}",
                                   name=f"w2aug{k}")
                         for k in range(H * HC // P)]
                for k in range(H * HC // P):
                    nc.sync.dma_start(w2aug[k][:, 0:F], c2w[k * P:(k + 1) * P, :])
                    tmp = sbC.tile([P, F], dt.float32, tag="w2tmp")
                    nc.vector.tensor_mul(tmp[:], w2aug[k][:, 0:F], c2a_bc[:, 0:F])
                    nc.vector.reduce_sum(w2aug[k][:, F:F + 1], tmp[:],
                                         axis=mybir.AxisListType.X)
                    nc.vector.tensor_mul(tmp[:], w2aug[k][:, 0:F], c2a_bc[:, F:2 * F])
                    nc.vector.reduce_sum(w2aug[k][:, F + 1:F + 2], tmp[:],
                                         axis=mybir.AxisListType.X)
                if debug:
                    for d in range(2):
                        dd = sbC.tile([P, H], dt.float32, tag="dendbg")
                        nc.vector.tensor_copy(dd[:], ps_den[d][:])
                        nc.sync.dma_start(dbg["den_dbg"][d * P:(d + 1) * P, :], dd[:])
                        dm = sbC.tile([P, H * HC], dt.float32, tag="msumdbg")
                        nc.vector.tensor_copy(dm[:], ps_msg[d][:])
                        nc.sync.dma_start(dbg["msum_dbg"][d * P:(d + 1) * P, :], dm[:])
                h1_keep = [keep.tile([P, H * HC], dt.float16, tag=f"h1k{d}",
                                     name=f"h1k{d}")
                           for d in range(2)]
                for d in range(2):
                    denr = sbC.tile([P, H], dt.float32, tag="denr")
                    nc.vector.reciprocal(denr[:], ps_den[d][:])
                    h1_t = h1_keep[d]
                    for h in range(H):
                        nc.vector.scalar_tensor_tensor(
                            h1_t[:, h * HC:(h + 1) * HC],
                            ps_msg[d][:, h * HC:(h + 1) * HC],
                            denr[:, h:h + 1],
                            c1b_bc[:, h * HC:(h + 1) * HC],
                            op0=ALU.mult, op1=ALU.add)
                    # elu = relu(x) + exp(min(x,0)) - 1
                    relu_t = sbC.tile([P, H * HC], dt.float32, tag="elu_r")
                    nc.scalar.activation(relu_t[:], h1_t[:], AF.Relu)
                    nc.vector.tensor_scalar_min(h1_t[:], h1_t[:], 0.0)
                    nc.scalar.activation(h1_t[:], h1_t[:], AF.Exp)
                    nc.vector.scalar_tensor_tensor(h1_t[:], h1_t[:], -1.0,
                                                   relu_t[:],
                                                   op0=ALU.add, op1=ALU.add)
                    if debug:
                        nc.sync.dma_start(dbg["h1_dbg"][d * P:(d + 1) * P, :], h1_t[:])
                esC.close()
                esD = ExitStack()
                sbD = esD.enter_context(tc.tile_pool(name="sbD", bufs=2))
                psD = esD.enter_context(tc.tile_pool(name="psD", bufs=1, space="PSUM"))
                w2aug_h = [sbD.tile([P, XP2W], dt.float16, tag=f"w2augh{k}", bufs=1,
                                    name=f"w2augh{k}")
                           for k in range(H * HC // P)]
                for k in range(H * HC // P):
                    nc.vector.tensor_copy(w2aug_h[k][:], w2aug[k][:])
                for d in range(2):
                    h1_t = h1_keep[d]
                    ps_xp2 = psD.tile([P, XP2W], dt.float32, space="PSUM", tag="ps_xp2")
                    for k in range(H * HC // P):
                        ps_h1t = psD.tile([P, P], dt.float16, space="PSUM",
                                          tag="ps_xth", bufs=2)
                        nc.tensor.transpose(ps_h1t[:], h1_t[:, k * P:(k + 1) * P],
                                            ident_h[:])
                        h1T = sb.tile([P, P], dt.float16, tag="xTh")
                        nc.vector.tensor_copy(h1T[:], ps_h1t[:])
                        nc.tensor.matmul(ps_xp2[:], lhsT=h1T[:], rhs=w2aug_h[k][:],
                                         start=(k == 0), stop=(k == H * HC // P - 1))
                    xp2_t = sbD.tile([P, XP2W], dt.float16, tag="xp2")
                    nc.vector.tensor_copy(xp2_t[:], ps_xp2[:])
                    nc.sync.dma_start(xp2_in[d * P:(d + 1) * P, :], xp2_t[:])
                esD.close()
                nc.gpsimd.collective_compute(
                    "AllGather", ALU.bypass, replica_groups=RG,
                    ins=[xp2_in[:]], outs=[xp2_dram[:]])
                if debug:
                    for nt in range(N // P):
                        dtmp = sb.tile([P, XP2W], dt.float32, tag="dbg1")
                        nc.sync.dma_start(dtmp[:], xp2_dram[nt * P:(nt + 1) * P, :])
                        nc.sync.dma_start(dbg["xp2_dbg"][nt * P:(nt + 1) * P, :],
                                          dtmp[:])

            if stage >= 3:
                # ============ conv2 aggregation ============
                esE = ExitStack()
                sbE = esE.enter_context(tc.tile_pool(name="sbE", bufs=2))
                psE = esE.enter_context(tc.tile_pool(name="psE", bufs=1, space="PSUM"))
                ps_m2 = [psE.tile([P, F + 1], dt.float32, space="PSUM",
                                  tag=f"ps_m2{d}", name=f"ps_m2{d}")
                         for d in range(2)]
                for t in range(nt_g):
                    oh_t = sbE.tile([P, NCHUNK], dt.float16, tag="oh2", bufs=4)
                    nc.sync.dma_start(oh_t[:], g_oh[t * P:(t + 1) * P, :])
                    xs2 = sbE.tile([P, XP2W], dt.float16, tag="xs2", bufs=4)
                    nc.gpsimd.indirect_dma_start(
                        out=xs2[:], out_offset=None, in_=xp2_dram[:],
                        in_offset=bass.IndirectOffsetOnAxis(
                            ap=gsrc_sb[:, t:t + 1], axis=0))
                    xd2 = sbE.tile([P, XP2W], dt.float16, tag="xd2", bufs=4)
                    nc.gpsimd.indirect_dma_start(
                        out=xd2[:], out_offset=None, in_=xp2_dram[:],
                        in_offset=bass.IndirectOffsetOnAxis(
                            ap=gdst_sb[:, t:t + 1], axis=0))
                    alpha2 = sbE.tile([P, 1], dt.float32, tag="alpha2")
                    nc.vector.tensor_add(alpha2[:], xs2[:, F:F + 1],
                                         xd2[:, F + 1:F + 2])
                    nc.vector.scalar_tensor_tensor(alpha2[:], alpha2[:], 0.2, alpha2[:],
                                                   op0=ALU.mult, op1=ALU.max)
                    ex2 = sbE.tile([P, 1], dt.float32, tag="ex2")
                    nc.scalar.activation(ex2[:], alpha2[:], AF.Exp)
                    nc.vector.tensor_mul(ex2[:], ex2[:], valid_t[t][:])
                    msg2 = sbE.tile([P, F + 1], dt.float16, tag="msg2")
                    nc.vector.tensor_tensor(msg2[:, 0:F], xs2[:, 0:F],
                                            ex2[:].to_broadcast([P, F]), op=ALU.mult)
                    nc.vector.tensor_copy(msg2[:, F:F + 1], ex2[:])
                    for d in range(2):
                        lhsT = oh_t[:, d * P:(d + 1) * P]
                        st, sp = (t == 0), (t == nt_g - 1)
                        nc.tensor.matmul(ps_m2[d][:], lhsT=lhsT, rhs=msg2[:],
                                         start=st, stop=sp)
                c2b_bc = sbE.tile([P, F], dt.float32, tag="c2bbc", bufs=1)
                nc.sync.dma_start(c2b_bc[:], bcast(c2b, F))
                for d in range(2):
                    d2r = sbE.tile([P, 1], dt.float32, tag="d2r")
                    nc.vector.reciprocal(d2r[:], ps_m2[d][:, F:F + 1])
                    h2_t = sbE.tile([P, F], dt.float16, tag="h2")
                    nc.vector.tensor_tensor(h2_t[:], ps_m2[d][:, 0:F],
                                            d2r[:].to_broadcast([P, F]), op=ALU.mult)
                    nc.vector.tensor_add(h2_t[:], h2_t[:], c2b_bc[:])
                    nc.sync.dma_start(h2_in[d * P:(d + 1) * P, :], h2_t[:])
                esE.close()
                nc.gpsimd.collective_compute(
                    "AllGather", ALU.bypass, replica_groups=RG,
                    ins=[h2_in[:]], outs=[h2_dram[:]])
                if debug:
                    for nt in range(N // P):
                        dtmp2 = sb.tile([P, F], dt.float32, tag="dbg2")
                        nc.sync.dma_start(dtmp2[:], h2_dram[nt * P:(nt + 1) * P, :])
                        nc.sync.dma_start(dbg["h2_dbg"][nt * P:(nt + 1) * P, :],
                                          dtmp2[:])

            if stage >= 4:
                # ============ edge MLP ============
                msrc_sb = single.tile([P, nt_m], dt.int32)
                nc.sync.dma_start(msrc_sb[:], m_src.ap().rearrange("(t p) -> p t", p=P))
                mdst_sb = single.tile([P, nt_m], dt.int32)
                nc.sync.dma_start(mdst_sb[:], m_dst.ap().rearrange("(t p) -> p t", p=P))
                mlidx_sb = single.tile([P, nt_m], dt.int32)
                nc.sync.dma_start(mlidx_sb[:], m_lidx.ap().rearrange("(t p) -> p t", p=P))
                m1w_t = [keep.tile([P, HC], dt.float32, tag=f"m1w{k}", name=f"m1w{k}")
                         for k in range(4 * F // P)]
                m1w_h = [keep.tile([P, HC], dt.float16, tag=f"m1wh{k}",
                                   name=f"m1wh{k}")
                         for k in range(4 * F // P)]
                for k in range(4 * F // P):
                    nc.sync.dma_start(m1w_t[k][:], m1w[k * P:(k + 1) * P, :])
                    nc.vector.tensor_copy(m1w_h[k][:], m1w_t[k][:])
                m1b_bc = single.tile([P, HC], dt.float32)
                nc.sync.dma_start(m1b_bc[:], bcast(m1b, HC))
                mask_f = [keep.tile([P, 1], dt.float32, tag=f"maskf{t}",
                                    name=f"maskf{t}")
                          for t in range(nt_m)]
                mask_u8 = [keep.tile([P, 1], dt.uint8, tag=f"masku{t}",
                                     name=f"masku{t}")
                           for t in range(nt_m)]
                z1_t = [keep.tile([P, HC], dt.float32, tag=f"z1_{t}", name=f"z1_{t}")
                        for t in range(nt_m)]
                esF = ExitStack()
                sbF = esF.enter_context(tc.tile_pool(name="sbF", bufs=2))
                psF = esF.enter_context(tc.tile_pool(name="psF", bufs=1, space="PSUM"))
                ps_s1 = psF.tile([1, HC], dt.float32, space="PSUM", tag="ps_s1")
                ps_q1 = psF.tile([1, HC], dt.float32, space="PSUM", tag="ps_q1")
                ps_cnt = psF.tile([1, 1], dt.float32, space="PSUM", tag="ps_cnt")
                for t in range(nt_m):
                    xi = sbF.tile([P, F], dt.float16, tag="xi", bufs=4)
                    nc.gpsimd.indirect_dma_start(
                        out=xi[:], out_offset=None, in_=h2_dram[:],
                        in_offset=bass.IndirectOffsetOnAxis(
                            ap=msrc_sb[:, t:t + 1], axis=0))
                    xj = sbF.tile([P, F], dt.float16, tag="xj", bufs=4)
                    nc.gpsimd.indirect_dma_start(
                        out=xj[:], out_offset=None, in_=h2_dram[:],
                        in_offset=bass.IndirectOffsetOnAxis(
                            ap=mdst_sb[:, t:t + 1], axis=0))
                    lg = sbF.tile([P, 1], dt.float32, tag="mlg", bufs=6)
                    nc.gpsimd.indirect_dma_start(
                        out=lg[:], out_offset=None, in_=lext_dram[:],
                        in_offset=bass.IndirectOffsetOnAxis(
                            ap=mlidx_sb[:, t:t + 1], axis=0))
                    nc.vector.tensor_scalar(mask_f[t][:], lg[:], 0.0, None,
                                            op0=ALU.is_gt)
                    nc.vector.tensor_copy(mask_u8[t][:], mask_f[t][:])
                    dsub = sbF.tile([P, F], dt.float16, tag="dsub")
                    nc.vector.tensor_sub(dsub[:], xi[:], xj[:])
                    nc.scalar.activation(dsub[:], dsub[:], AF.Abs)
                    pmul = sbF.tile([P, F], dt.float16, tag="pmul")
                    nc.vector.tensor_mul(pmul[:], xi[:], xj[:])
                    ps_z1 = psF.tile([P, HC], dt.float32, space="PSUM", tag="ps_z1")
                    for pi, piece in enumerate((xi, xj, dsub, pmul)):
                        for hf in range(2):
                            ps_t = psF.tile([P, P], dt.float16, space="PSUM",
                                            tag="ps_xth", bufs=2)
                            nc.tensor.transpose(ps_t[:], piece[:, hf * P:(hf + 1) * P],
                                                ident_h[:])
                            efT = sb.tile([P, P], dt.float16, tag="xTh")
                            nc.vector.tensor_copy(efT[:], ps_t[:])
                            k = pi * 2 + hf
                            nc.tensor.matmul(ps_z1[:], lhsT=efT[:],
                                             rhs=m1w_h[k][:],
                                             start=(k == 0), stop=(k == 7))
                    nc.vector.tensor_add(z1_t[t][:], ps_z1[:], m1b_bc[:])
                    if debug:
                        nc.sync.dma_start(dbg["z1_dbg"][t * P:(t + 1) * P, :],
                                          z1_t[t][:])
                    zsq = sbF.tile([P, HC], dt.float32, tag="zsq")
                    nc.vector.tensor_mul(zsq[:], z1_t[t][:], z1_t[t][:])
                    st, sp = (t == 0), (t == nt_m - 1)
                    nc.tensor.matmul(ps_s1[:], lhsT=mask_f[t][:], rhs=z1_t[t][:],
                                     start=st, stop=sp)
                    nc.tensor.matmul(ps_q1[:], lhsT=mask_f[t][:], rhs=zsq[:],
                                     start=st, stop=sp)
                    nc.tensor.matmul(ps_cnt[:], lhsT=mask_f[t][:], rhs=mask_f[t][:],
                                     start=st, stop=sp)
                # pack stats1, AllReduce
                s_sb = sbF.tile([1, HC], dt.float32, tag="stat")
                nc.vector.tensor_copy(s_sb[:], ps_s1[:])
                nc.sync.dma_start(st1_in[None, 0:HC], s_sb[:])
                q_sb = sbF.tile([1, HC], dt.float32, tag="stat")
                nc.vector.tensor_copy(q_sb[:], ps_q1[:])
                nc.sync.dma_start(st1_in[None, HC:2 * HC], q_sb[:])
                c_sb = sbF.tile([1, 1], dt.float32, tag="statc")
                nc.vector.tensor_copy(c_sb[:], ps_cnt[:])
                nc.sync.dma_start(st1_in[None, 2 * HC:2 * HC + 1], c_sb[:])
                zpad = sbF.tile([1, 7], dt.float32, tag="statz")
                nc.vector.memset(zpad[:], 0.0)
                nc.sync.dma_start(st1_in[None, 2 * HC + 1:520], zpad[:])
                esF.close()
                nc.gpsimd.collective_compute(
                    "AllReduce", ALU.add, replica_groups=RG,
                    ins=[st1_in[:]], outs=[st1_out[:]])
                if debug:
                    dstat = sb.tile([1, 520], dt.float32, tag="dbg3")
                    nc.sync.dma_start(dstat[:], st1_out[None, :])
                    nc.sync.dma_start(dbg["st1_dbg"][None, :], dstat[:])

                esG = ExitStack()
                sbG = esG.enter_context(tc.tile_pool(name="sbG", bufs=2))
                psG = esG.enter_context(tc.tile_pool(name="psG", bufs=1, space="PSUM"))

                def bn_rows(st_out, nch, g_in, b_in, row_off):
                    s_row = sbG.tile([1, nch], dt.float32, tag="bnrow_s_row", name="s_row")
                    nc.sync.dma_start(s_row[:], st_out[None, 0:nch])
                    q_row = sbG.tile([1, nch], dt.float32, tag="bnrow_q_row", name="q_row")
                    nc.sync.dma_start(q_row[:], st_out[None, nch:2 * nch])
                    cnt_row = sbG.tile([1, 1], dt.float32, tag="bnrow_cnt_row", name="cnt_row")
                    nc.sync.dma_start(cnt_row[:], st1_out[None, 2 * HC:2 * HC + 1])
                    nc.vector.tensor_scalar_max(cnt_row[:], cnt_row[:], 1.0)
                    cr = sbG.tile([1, 1], dt.float32, tag="bnrow_cr", name="cr")
                    nc.vector.reciprocal(cr[:], cnt_row[:])
                    mean = sbG.tile([1, nch], dt.float32, tag="bnrow_mean", name="mean")
                    nc.vector.tensor_tensor(mean[:], s_row[:],
                                            cr[:].to_broadcast([1, nch]), op=ALU.mult)
                    var = sbG.tile([1, nch], dt.float32, tag="bnrow_var", name="var")
                    nc.vector.tensor_tensor(var[:], q_row[:],
                                            cr[:].to_broadcast([1, nch]), op=ALU.mult)
                    msq = sbG.tile([1, nch], dt.float32, tag="bnrow_msq", name="msq")
                    nc.vector.tensor_mul(msq[:], mean[:], mean[:])
                    nc.vector.tensor_sub(var[:], var[:], msq[:])
                    nc.vector.tensor_scalar_add(var[:], var[:], 1e-5)
                    nc.scalar.activation(var[:], var[:], AF.Sqrt)
                    rstd = sbG.tile([1, nch], dt.float32, tag="bnrow_rstd", name="rstd")
                    nc.vector.reciprocal(rstd[:], var[:])
                    g_row = sbG.tile([1, nch], dt.float32, tag="bnrow_g_row", name="g_row")
                    nc.sync.dma_start(g_row[:], g_in[None, :])
                    b_row = sbG.tile([1, nch], dt.float32, tag="bnrow_b_row", name="b_row")
                    nc.sync.dma_start(b_row[:], b_in[None, :])
                    gs = sbG.tile([1, nch], dt.float32, tag="bnrow_gs", name="gs")
                    nc.vector.tensor_mul(gs[:], g_row[:], rstd[:])
                    gb = sbG.tile([1, nch], dt.float32, tag="bnrow_gb", name="gb")
                    nc.vector.tensor_mul(gb[:], mean[:], gs[:])
                    nc.vector.tensor_sub(gb[:], b_row[:], gb[:])
                    nc.sync.dma_start(row_dram[None, row_off:row_off + nch], gs[:])
                    nc.sync.dma_start(row_dram[None, row_off + nch:row_off + 2 * nch],
                                      gb[:])

                bn_rows(st1_out, HC, bn1g, bn1b, 0)
                gs1_bc = single.tile([P, HC], dt.float32)
                nc.sync.dma_start(gs1_bc[:], bcast_ap(row_dram, HC, 0))
                gb1_bc = single.tile([P, HC], dt.float32)
                nc.sync.dma_start(gb1_bc[:], bcast_ap(row_dram, HC, HC))
                m2w_t = [keep.tile([P, HC2], dt.float32, tag=f"m2w{k}", name=f"m2w{k}")
                         for k in range(HC // P)]
                m2w_h = [keep.tile([P, HC2], dt.float16, tag=f"m2wh{k}",
                                   name=f"m2wh{k}")
                         for k in range(HC // P)]
                for k in range(HC // P):
                    nc.sync.dma_start(m2w_t[k][:], m2w[k * P:(k + 1) * P, :])
                    nc.vector.tensor_copy(m2w_h[k][:], m2w_t[k][:])
                m2b_bc = single.tile([P, HC2], dt.float32)
                nc.sync.dma_start(m2b_bc[:], bcast(m2b, HC2))
                z2_t = [keep.tile([P, HC2], dt.float32, tag=f"z2_{t}", name=f"z2_{t}")
                        for t in range(nt_m)]
                ps_s2 = psG.tile([1, HC2], dt.float32, space="PSUM", tag="ps_s2")
                ps_q2 = psG.tile([1, HC2], dt.float32, space="PSUM", tag="ps_q2")
                for t in range(nt_m):
                    zn = sbG.tile([P, HC], dt.float16, tag="zn")
                    nc.vector.tensor_mul(zn[:], z1_t[t][:], gs1_bc[:])
                    nc.vector.tensor_add(zn[:], zn[:], gb1_bc[:])
                    nc.scalar.activation(zn[:], zn[:], AF.Relu)
                    ps_z2 = psG.tile([P, HC2], dt.float32, space="PSUM", tag="ps_z2")
                    for k in range(HC // P):
                        ps_t = psG.tile([P, P], dt.float16, space="PSUM",
                                        tag="ps_xth", bufs=2)
                        nc.tensor.transpose(ps_t[:], zn[:, k * P:(k + 1) * P],
                                            ident_h[:])
                        znT = sb.tile([P, P], dt.float16, tag="xTh")
                        nc.vector.tensor_copy(znT[:], ps_t[:])
                        nc.tensor.matmul(ps_z2[:], lhsT=znT[:], rhs=m2w_h[k][:],
                                         start=(k == 0), stop=(k == HC // P - 1))
                    nc.vector.tensor_add(z2_t[t][:], ps_z2[:], m2b_bc[:])
                    zsq2 = sbG.tile([P, HC2], dt.float32, tag="zsq2")
                    nc.vector.tensor_mul(zsq2[:], z2_t[t][:], z2_t[t][:])
                    st, sp = (t == 0), (t == nt_m - 1)
                    nc.tensor.matmul(ps_s2[:], lhsT=mask_f[t][:], rhs=z2_t[t][:],
                                     start=st, stop=sp)
                    nc.tensor.matmul(ps_q2[:], lhsT=mask_f[t][:], rhs=zsq2[:],
                                     start=st, stop=sp)
                s2_sb = sbG.tile([1, HC2], dt.float32, tag="stat2")
                nc.vector.tensor_copy(s2_sb[:], ps_s2[:])
                nc.sync.dma_start(st2_in[None, 0:HC2], s2_sb[:])
                q2_sb = sbG.tile([1, HC2], dt.float32, tag="stat2")
                nc.vector.tensor_copy(q2_sb[:], ps_q2[:])
                nc.sync.dma_start(st2_in[None, HC2:2 * HC2], q2_sb[:])
                nc.gpsimd.collective_compute(
                    "AllReduce", ALU.add, replica_groups=RG,
                    ins=[st2_in[:]], outs=[st2_out[:]])
                bn_rows(st2_out, HC2, bn2g, bn2b, 2 * HC)
                gs2_bc = single.tile([P, HC2], dt.float32)
                nc.sync.dma_start(gs2_bc[:], bcast_ap(row_dram, HC2, 2 * HC))
                gb2_bc = single.tile([P, HC2], dt.float32)
                nc.sync.dma_start(gb2_bc[:], bcast_ap(row_dram, HC2, 2 * HC + HC2))
                m3w_sb = single.tile([P, 1], dt.float32)
                nc.sync.dma_start(m3w_sb[:], m3w[:, :])
                m3w_h = single.tile([P, 1], dt.float16)
                nc.vector.tensor_copy(m3w_h[:], m3w_sb[:])
                m3w_h = single.tile([P, 1], dt.float16)
                nc.vector.tensor_copy(m3w_h[:], m3w_sb[:])

                m3b_bc = single.tile([P, 1], dt.float32)
                nc.sync.dma_start(m3b_bc[:], bcast(m3b, 1))
                neg25 = single.tile([P, 1], dt.float32)
                nc.vector.memset(neg25[:], -2.5)
                for t in range(nt_m):
                    zn2 = sbG.tile([P, HC2], dt.float16, tag="zn2")
                    nc.vector.tensor_mul(zn2[:], z2_t[t][:], gs2_bc[:])
                    nc.vector.tensor_add(zn2[:], zn2[:], gb2_bc[:])
                    nc.scalar.activation(zn2[:], zn2[:], AF.Relu)
                    ps_t = psG.tile([P, P], dt.float16, space="PSUM",
                                    tag="ps_xth", bufs=2)
                    nc.tensor.transpose(ps_t[:], zn2[:], ident_h[:])
                    znT2 = sb.tile([P, P], dt.float16, tag="xTh")
                    nc.vector.tensor_copy(znT2[:], ps_t[:])
                    ps_sc = psG.tile([P, 1], dt.float32, space="PSUM", tag="ps_sc")
                    nc.tensor.matmul(ps_sc[:], lhsT=znT2[:], rhs=m3w_h[:],
                                     start=True, stop=True)
                    score = sbG.tile([P, 1], dt.float32, tag="score")
                    nc.vector.tensor_add(score[:], ps_sc[:], m3b_bc[:])
                    sel = sbG.tile([P, 1], dt.float32, tag="sel")
                    nc.vector.select(sel[:], mask_u8[t][:], score[:], neg25[:])
                    nc.scalar.activation(sel[:], sel[:], AF.Sigmoid)
                    nc.sync.dma_start(score_out[t * P:(t + 1) * P][:, None], sel[:])
                esG.close()

    nc.compile()
    return nc


def kernel(**inputs):
    inputs = {k: np.asarray(v) for k, v in inputs.items()}
    src = inputs["edge_index"][0].astype(np.int64)
    dst = inputs["edge_index"][1].astype(np.int64)

    # --- edge partition by dst chunk (GAT set includes self loops) ---
    all_src = np.concatenate([src, np.arange(N, dtype=np.int64)])
    all_dst = np.concatenate([dst, np.arange(N, dtype=np.int64)])
    lidx_all = np.concatenate(
        [np.arange(E, dtype=np.int64), np.full(N, E, dtype=np.int64)])
    chunk_g = all_dst // NCHUNK
    gids = [np.where(chunk_g == c)[0] for c in range(NCORES)]
    nt_g = int(np.ceil(max(len(i) for i in gids) / P))
    pad_g = nt_g * P
    chunk_m = dst // NCHUNK
    mids = [np.where(chunk_m == c)[0] for c in range(NCORES)]
    nt_m = int(np.ceil(max(len(i) for i in mids) / P))
    pad_m = nt_m * P

    key = (nt_g, nt_m, DEBUG)
    if key not in _cache:
        _cache[key] = _build(nt_g, nt_m, debug=DEBUG)
    nc = _cache[key]

    shared = dict(
        x=inputs["x"],
        sent_emb=inputs["sent_emb"],
        elp=inputs["edge_logits_param"],
        fc0_w=inputs["fc0_w"], fc0_b=inputs["fc0_b"],
        fc1_w=inputs["fc1_w"], fc1_b=inputs["fc1_b"],
        conv1_W=inputs["conv1_W"],
        conv1_a=np.concatenate([inputs["conv1_asrc"].reshape(-1),
                                inputs["conv1_adst"].reshape(-1)]),
        conv1_b=inputs["conv1_b"],
        conv2_W=inputs["conv2_W"],
        conv2_a=np.concatenate([inputs["conv2_asrc"].reshape(-1),
                                inputs["conv2_adst"].reshape(-1)]),
        conv2_b=inputs["conv2_b"],
        mlp1_w=inputs["mlp1_w"], mlp1_b=inputs["mlp1_b"],
        bn1_g=inputs["bn1_g"], bn1_b=inputs["bn1_b"],
        mlp2_w=inputs["mlp2_w"], mlp2_b=inputs["mlp2_b"],
        bn2_g=inputs["bn2_g"], bn2_b=inputs["bn2_b"],
        mlp3_w=inputs["mlp3_w"], mlp3_b=inputs["mlp3_b"],
        fc2_b=inputs["fc2_b"],
    )
    shared = {k: np.ascontiguousarray(v, dtype=np.float32)
              for k, v in shared.items()}

    fcl_w, fce_w, fc2_w = inputs["fcl_w"], inputs["fce_w"], inputs["fc2_w"]
    in_maps = []
    for c in range(NCORES):
        gi = gids[c]
        mi = mids[c]
        gsrc = np.zeros(pad_g, np.int32); gsrc[:len(gi)] = all_src[gi]
        gdst = np.zeros(pad_g, np.int32); gdst[:len(gi)] = all_dst[gi]
        glidx = np.full(pad_g, E + 1, np.int32); glidx[:len(gi)] = lidx_all[gi]
        goh = np.zeros((pad_g, NCHUNK), np.float16)
        goh[np.arange(len(gi)), all_dst[gi] - c * NCHUNK] = 1.0
        msrc = np.zeros(pad_m, np.int32); msrc[:len(mi)] = src[mi]
        mdst = np.zeros(pad_m, np.int32); mdst[:len(mi)] = dst[mi]
        mlidx = np.full(pad_m, E + 1, np.int32); mlidx[:len(mi)] = mi
        m = dict(shared)
        m.update(
            fclw_sh=np.ascontiguousarray(fcl_w[:, c * ECH:(c + 1) * ECH],
                                         dtype=np.float32),
            fclb_sh=np.ascontiguousarray(inputs["fcl_b"][c * ECH:(c + 1) * ECH],
                                         dtype=np.float32),
            fcew_sh=np.ascontiguousarray(fce_w[:, c * ECH:(c + 1) * ECH],
                                         dtype=np.float32),
            fceb_sh=np.ascontiguousarray(inputs["fce_b"][c * ECH:(c + 1) * ECH],
                                         dtype=np.float32),
            fc2w_sh=np.ascontiguousarray(np.concatenate(
                [fc2_w[c * ECH:(c + 1) * ECH],
                 fc2_w[E + c * ECH:E + (c + 1) * ECH]], axis=0),
                dtype=np.float32),
            g_src=gsrc, g_dst=gdst, g_lidx=glidx, g_oh=goh,
            m_src=msrc, m_dst=mdst, m_lidx=mlidx,
        )
        in_maps.append(m)

    global last_results, last_in_maps
    last_in_maps = in_maps
    res = run_bass_kernel_spmd(nc, in_maps, core_ids=list(range(NCORES)),
                               trace=TRACE)
    last_results = res
    orig = res.results[0]["orig_out"].reshape(E).astype(np.float32)
    sig = np.empty(E, np.float32)
    for c in range(NCORES):
        mi = mids[c]
        sig[mi] = res.results[c]["score_out"].reshape(pad_m)[:len(mi)]
    return sig, orig

